# revision 8
# baseline (speedup 1.0000x reference)
"""Trainium2 Bass kernel for nn_MeshEdgeUpdate (gnn_message_passing).

Sharding: edge dim (m_enum) split across 8 cores (padded 327660 -> 327680,
40960 edges/core); mx replicated. Per-core: indirect-DMA gather of mx rows
(cast f32->bf16 in DMA), PE-transpose to feature-major, 3-layer MLP in bf16
on the tensor engine (fp32 PSUM accumulation), LayerNorm stats accumulated
via scalar-engine accum_out, AllReduce of [sum, sumsq] across cores, then a
second pass normalizes + residual-adds and writes the output.
"""
import json

import numpy as np

import concourse.bass as bass
import concourse.mybir as mybir
import concourse.tile as tile
from concourse.bass_utils import run_bass_kernel_spmd
from concourse.masks import make_identity

# problem dims (hardcoded per contract)
B = 1
MN = 40962
ME = 327660
MEMB = EEMB = 256
H1, H2 = 512, 64
EPS = 1e-5
NCORES = 8

# tiling
CCH = 512          # compute chunk (edges)
KJ = CCH // 128    # 4 sub-blocks of 128 edges

F32 = mybir.dt.float32
BF16 = mybir.dt.bfloat16
I32 = mybir.dt.int32


def _split_multi_waits(bir_json: bytes) -> bytes:
    """This walrus build allows only one on_wait per instruction; hoist the
    extras into standalone EventSemaphore instructions."""
    j = json.loads(bir_json)
    counter = [0]

    def fix_block(block):
        out = []
        for inst in block.get("instructions", []):
            si = inst.get("sync_info") or {}
            waits = si.get("on_wait") or []
            if len(waits) > 1:
                for w in waits[:-1]:
                    counter[0] += 1
                    ev = {
                        "name": f"WSPLIT-{counter[0]}",
                        "opcode": "EventSemaphore",
                        "engine": inst.get("engine"),
                        "ins": [],
                        "outs": [],
                        "sync_info": {"on_wait": [w], "on_update": []},
                    }
                    if "debug" in inst:
                        ev["debug"] = inst["debug"]
                    out.append(ev)
                si["on_wait"] = [waits[-1]]
            out.append(inst)
        block["instructions"] = out
        for b in block.get("blocks", []):
            fix_block(b)

    for f in j.get("functions", []):
        for b in f.get("blocks", []):
            fix_block(b)
    return json.dumps(j).encode()


def build_nc(e_loc: int, mn: int, gch: int, p2ch: int, ln_affine: bool,
             n_cores: int = NCORES):
    """Build the per-core Bass kernel.

    e_loc: edges per core (multiple of gch); gch: gather chunk (multiple of
    CCH); p2ch: pass-2 chunk (multiple of 128, divides e_loc).
    """
    assert e_loc % gch == 0 and gch % CCH == 0 and e_loc % p2ch == 0
    ng = e_loc // gch          # gather chunks
    cpg = gch // CCH           # compute chunks per gather chunk
    nch = e_loc // CCH         # compute chunks
    np2 = e_loc // p2ch        # pass-2 chunks
    kg = gch // 128            # idx slots per gather chunk
    kp2 = p2ch // 128
    n_total = float(n_cores * e_loc * EEMB)

    nc = bass.Bass("TRN2", num_devices=n_cores)

    mx = nc.dram_tensor("mx", [mn, MEMB], F32, kind="ExternalInput")
    mex = nc.dram_tensor("mex", [e_loc, EEMB], F32, kind="ExternalInput")
    rows = nc.dram_tensor("rows", [ng, 128, kg], I32, kind="ExternalInput")
    cols = nc.dram_tensor("cols", [ng, 128, kg], I32, kind="ExternalInput")
    w1 = nc.dram_tensor("w1", [2 * MEMB + EEMB, H1], F32, kind="ExternalInput")
    b1 = nc.dram_tensor("b1", [H1], F32, kind="ExternalInput")
    w2 = nc.dram_tensor("w2", [H1, H2], F32, kind="ExternalInput")
    b2 = nc.dram_tensor("b2", [H2], F32, kind="ExternalInput")
    w3 = nc.dram_tensor("w3", [H2, EEMB], F32, kind="ExternalInput")
    b3 = nc.dram_tensor("b3", [EEMB], F32, kind="ExternalInput")
    if ln_affine:
        lnw = nc.dram_tensor("lnw", [e_loc, EEMB], F32, kind="ExternalInput")
        lnb = nc.dram_tensor("lnb", [e_loc, EEMB], F32, kind="ExternalInput")
    out = nc.dram_tensor("out", [e_loc, EEMB], F32, kind="ExternalOutput")
    h3sp = nc.dram_tensor("h3sp", [e_loc, EEMB], BF16)  # internal spill

    # tiled DRAM views: edge e = chunk*W + j*128 + p  ->  [chunk, p, j, f]
    mex_g = mex[:].rearrange("(g j p) f -> g p j f", j=kg, p=128)
    mex_2 = mex[:].rearrange("(q j p) f -> q p j f", j=kp2, p=128)
    h3_c = h3sp[:].rearrange("(c j p) f -> c p j f", j=KJ, p=128)
    h3_2 = h3sp[:].rearrange("(q j p) f -> q p j f", j=kp2, p=128)
    out_2 = out[:].rearrange("(q j p) f -> q p j f", j=kp2, p=128)
    if ln_affine:
        lnw_2 = lnw[:].rearrange("(q j p) f -> q p j f", j=kp2, p=128)
        lnb_2 = lnb[:].rearrange("(q j p) f -> q p j f", j=kp2, p=128)
    w1_v = w1[:].rearrange("(k p) h -> p k h", p=128)   # [128, 6, 512]
    w2_v = w2[:].rearrange("(k p) h -> p k h", p=128)   # [128, 4, 64]
    b1_v = b1[:].rearrange("(m p) -> p m", p=128)       # [128, 4]
    b3_v = b3[:].rearrange("(m p) -> p m", p=128)       # [128, 2]

    with tile.TileContext(nc) as tc:
        with (
            tc.tile_pool(name="singles", bufs=1) as sg,
            tc.tile_pool(name="gather", bufs=2) as gp,
            tc.tile_pool(name="work", bufs=2) as wp,
            tc.tile_pool(name="p2", bufs=2) as p2p,
            tc.tile_pool(name="trps", bufs=2, space="PSUM") as trps,
            tc.tile_pool(name="h1ps", bufs=2, space="PSUM") as h1ps,
            tc.tile_pool(name="h23ps", bufs=2, space="PSUM") as h23ps,
            tc.tile_pool(name="tokps", bufs=2, space="PSUM") as tokps,
            tc.tile_pool(name="dram", bufs=1, space="DRAM") as dram,
        ):
            # ---- constants / weights (once) ----
            idn = sg.tile([128, 128], BF16)
            make_identity(nc, idn[:])
            w1b = sg.tile([128, 6, H1], BF16)
            nc.gpsimd.dma_start(out=w1b[:], in_=w1_v)      # casts f32->bf16
            w2b = sg.tile([128, 4, H2], BF16)
            nc.gpsimd.dma_start(out=w2b[:], in_=w2_v)
            w3b = sg.tile([H2, EEMB], BF16)
            nc.gpsimd.dma_start(out=w3b[:], in_=w3[:, :])
            b1t = sg.tile([128, H1 // 128], F32)
            nc.sync.dma_start(out=b1t[:], in_=b1_v)
            b2t = sg.tile([H2, 1], F32)
            nc.sync.dma_start(out=b2t[:], in_=b2[:, None])
            b3t = sg.tile([128, EEMB // 128], F32)
            nc.sync.dma_start(out=b3t[:], in_=b3_v)
            epst = sg.tile([128, 1], F32)
            nc.vector.memset(epst[:], EPS)
            onest = sg.tile([128, 1], F32)
            nc.vector.memset(onest[:], 1.0)
            ssum = sg.tile([128, nch * 2], F32)   # per-(chunk, m) act sums
            qsum = sg.tile([128, nch * 2], F32)   # per-(chunk, m) sq sums

            # ---- pass 1 ----
            for g in range(ng):
                rt = gp.tile([128, kg], I32, tag="rt")
                nc.sync.dma_start(out=rt[:], in_=rows[g])
                ct = gp.tile([128, kg], I32, tag="ct")
                nc.sync.dma_start(out=ct[:], in_=cols[g])
                gr = gp.tile([128, kg, MEMB], BF16, tag="gr")
                gc = gp.tile([128, kg, MEMB], BF16, tag="gc")
                # HW indirect DMA only honors one index per partition -> one
                # 128-row gather per j-slot.
                for jj in range(kg):
                    nc.gpsimd.indirect_dma_start(
                        out=gr[:, jj, :], out_offset=None, in_=mx[:],
                        in_offset=bass.IndirectOffsetOnAxis(
                            ap=rt[:, jj:jj + 1], axis=0))
                    nc.gpsimd.indirect_dma_start(
                        out=gc[:, jj, :], out_offset=None, in_=mx[:],
                        in_offset=bass.IndirectOffsetOnAxis(
                            ap=ct[:, jj:jj + 1], axis=0))
                mxt = gp.tile([128, kg, EEMB], BF16, tag="mxt")
                nc.gpsimd.dma_start(out=mxt[:], in_=mex_g[g])

                for cc in range(cpg):
                    c = g * cpg + cc
                    # x1T: 6 feature-major slices [128, CCH]
                    x1T = []
                    for s, (src, half) in enumerate(
                        [(gr, 0), (gr, 1), (gc, 0), (gc, 1), (mxt, 0), (mxt, 1)]
                    ):
                        ps = trps.tile([128, CCH], BF16, tag="trps")
                        for j in range(KJ):
                            jj = cc * KJ + j
                            nc.tensor.transpose(
                                out=ps[:, j * 128:(j + 1) * 128],
                                in_=src[:, jj, half * 128:(half + 1) * 128],
                                identity=idn[:])
                        sb = wp.tile([128, CCH], BF16, tag=f"x1T{s}")
                        nc.vector.tensor_copy(out=sb[:], in_=ps[:])
                        x1T.append(sb)

                    # W1 + silu -> h1T (4 slices)
                    h1sb = []
                    for m in range(4):
                        h1p = h1ps.tile([128, CCH], F32, tag="h1p")
                        for k in range(6):
                            nc.tensor.matmul(
                                out=h1p[:],
                                lhsT=w1b[:, k, m * 128:(m + 1) * 128],
                                rhs=x1T[k][:],
                                start=(k == 0), stop=(k == 5))
                        hs = wp.tile([128, CCH], BF16, tag=f"h1sb{m}")
                        nc.scalar.activation(
                            out=hs[:], in_=h1p[:],
                            func=mybir.ActivationFunctionType.Silu,
                            bias=b1t[:, m:m + 1], scale=1.0)
                        h1sb.append(hs)

                    # W2 + silu -> h2T [64, CCH]
                    h2p = h23ps.tile([H2, CCH], F32, tag="h23")
                    for k in range(4):
                        nc.tensor.matmul(
                            out=h2p[:], lhsT=w2b[:, k, :], rhs=h1sb[k][:],
                            start=(k == 0), stop=(k == 3))
                    h2sb = wp.tile([H2, CCH], BF16, tag="h2sb")
                    nc.scalar.activation(
                        out=h2sb[:], in_=h2p[:],
                        func=mybir.ActivationFunctionType.Silu,
                        bias=b2t[:, 0:1], scale=1.0)

                    # W3 + bias (+ stats) -> h3T (2 slices)
                    h3sb = []
                    for m in range(2):
                        h3p = h23ps.tile([128, CCH], F32, tag="h23")
                        nc.tensor.matmul(
                            out=h3p[:], lhsT=w3b[:, m * 128:(m + 1) * 128],
                            rhs=h2sb[:], start=True, stop=True)
                        hs = wp.tile([128, CCH], BF16, tag=f"h3sb{m}")
                        nc.scalar.activation(
                            out=hs[:], in_=h3p[:],
                            func=mybir.ActivationFunctionType.Identity,
                            bias=b3t[:, m:m + 1], scale=1.0,
                            accum_out=ssum[:, 2 * c + m:2 * c + m + 1])
                        sq = wp.tile([128, CCH], BF16, tag="sqscr")
                        nc.scalar.activation(
                            out=sq[:], in_=hs[:],
                            func=mybir.ActivationFunctionType.Square,
                            accum_out=qsum[:, 2 * c + m:2 * c + m + 1])
                        h3sb.append(hs)

                    # back to token-major and spill
                    h3tok = wp.tile([128, KJ, EEMB], BF16, tag="h3tok")
                    for j in range(KJ):
                        ps = tokps.tile([128, EEMB], BF16, tag="tokps")
                        for m in range(2):
                            nc.tensor.transpose(
                                out=ps[:, m * 128:(m + 1) * 128],
                                in_=h3sb[m][:, j * 128:(j + 1) * 128],
                                identity=idn[:])
                        nc.vector.tensor_copy(out=h3tok[:, j, :], in_=ps[:])
                    nc.sync.dma_start(out=h3_c[c], in_=h3tok[:])

            # ---- stats reduce + AllReduce ----
            st2 = sg.tile([128, 2], F32)
            nc.vector.reduce_sum(out=st2[:, 0:1], in_=ssum[:],
                                 axis=mybir.AxisListType.X)
            nc.vector.reduce_sum(out=st2[:, 1:2], in_=qsum[:],
                                 axis=mybir.AxisListType.X)
            stp = h23ps.tile([1, 2], F32, tag="h23")
            nc.tensor.matmul(out=stp[:], lhsT=onest[:], rhs=st2[:],
                             start=True, stop=True)
            stsb = sg.tile([1, 2], F32)
            nc.vector.tensor_copy(out=stsb[:], in_=stp[:])
            cc_in = dram.tile([1, 2], F32)
            cc_out = dram.tile([1, 2], F32)
            nc.sync.dma_start(out=cc_in[:], in_=stsb[:])
            nc.gpsimd.collective_compute(
                "AllReduce", mybir.AluOpType.add,
                replica_groups=[list(range(n_cores))],
                ins=[cc_in[:].opt()], outs=[cc_out[:].opt()])
            stbc = sg.tile([128, 2], F32)
            nc.sync.dma_start(out=stbc[:], in_=cc_out[0:1, :].to_broadcast([128, 2]))

            mu = sg.tile([128, 1], F32)
            nc.scalar.mul(out=mu[:], in_=stbc[:, 0:1], mul=1.0 / n_total)
            e2 = sg.tile([128, 1], F32)
            nc.scalar.mul(out=e2[:], in_=stbc[:, 1:2], mul=1.0 / n_total)
            mu2 = sg.tile([128, 1], F32)
            nc.vector.tensor_mul(out=mu2[:], in0=mu[:], in1=mu[:])
            var = sg.tile([128, 1], F32)
            nc.vector.tensor_tensor(out=var[:], in0=e2[:], in1=mu2[:],
                                    op=mybir.AluOpType.subtract)
            std = sg.tile([128, 1], F32)
            nc.scalar.activation(out=std[:], in_=var[:],
                                 func=mybir.ActivationFunctionType.Sqrt,
                                 bias=epst[:, 0:1], scale=1.0)
            rstd = sg.tile([128, 1], F32)
            nc.vector.reciprocal(out=rstd[:], in_=std[:])
            murs = sg.tile([128, 1], F32)
            nc.vector.tensor_mul(out=murs[:], in0=mu[:], in1=rstd[:])
            shift = sg.tile([128, 1], F32)
            nc.scalar.mul(out=shift[:], in_=murs[:], mul=-1.0)

            # ---- pass 2: normalize + residual ----
            for q in range(np2):
                h3c = p2p.tile([128, kp2, EEMB], BF16, tag="h3c")
                nc.sync.dma_start(out=h3c[:], in_=h3_2[q])
                nm = p2p.tile([128, kp2, EEMB], F32, tag="nm")
                nc.scalar.activation(
                    out=nm[:], in_=h3c[:],
                    func=mybir.ActivationFunctionType.Identity,
                    bias=shift[:, 0:1], scale=rstd[:, 0:1])
                if ln_affine:
                    lwt = p2p.tile([128, kp2, EEMB], F32, tag="lwt")
                    nc.sync.dma_start(out=lwt[:], in_=lnw_2[q])
                    lbt = p2p.tile([128, kp2, EEMB], F32, tag="lbt")
                    nc.sync.dma_start(out=lbt[:], in_=lnb_2[q])
                    nc.vector.tensor_mul(out=nm[:], in0=nm[:], in1=lwt[:])
                    nc.vector.tensor_add(out=nm[:], in0=nm[:], in1=lbt[:])
                mx2 = p2p.tile([128, kp2, EEMB], F32, tag="mx2")
                nc.sync.dma_start(out=mx2[:], in_=mex_2[q])
                ot = p2p.tile([128, kp2, EEMB], F32, tag="ot")
                nc.vector.tensor_add(out=ot[:], in0=nm[:], in1=mx2[:])
                nc.sync.dma_start(out=out_2[q], in_=ot[:])

    orig = nc.to_json_bytes
    nc.to_json_bytes = lambda: _split_multi_waits(orig())
    return nc


def _prep_host(inputs, e_loc, gch, n_cores=NCORES):
    """Slice/pad/permute the full inputs into per-core in_maps."""
    mx = np.ascontiguousarray(np.asarray(inputs["mx"], dtype=np.float32)[0])
    me_x = np.asarray(inputs["me_x"], dtype=np.float32)[0]
    me_i = np.asarray(inputs["me_i"]).astype(np.int32)
    me = me_x.shape[0]
    e_pad = e_loc * n_cores
    kg = gch // 128
    ng = e_loc // gch

    mex_p = np.zeros((e_pad, EEMB), dtype=np.float32)
    mex_p[:me] = me_x
    idx_p = np.zeros((2, e_pad), dtype=np.int32)
    idx_p[:, :me] = me_i

    w1 = np.asarray(inputs["W1"], dtype=np.float32)
    b1 = np.asarray(inputs["b1"], dtype=np.float32)
    w2 = np.asarray(inputs["W2"], dtype=np.float32)
    b2 = np.asarray(inputs["b2"], dtype=np.float32)
    w3 = np.asarray(inputs["W3"], dtype=np.float32)
    b3 = np.asarray(inputs["b3"], dtype=np.float32)
    ln_w = np.asarray(inputs["ln_w"], dtype=np.float32)
    ln_b = np.asarray(inputs["ln_b"], dtype=np.float32)
    ln_affine = not (np.all(ln_w == 1.0) and np.all(ln_b == 0.0))
    if ln_affine:
        lnw_p = np.ones((e_pad, EEMB), dtype=np.float32)
        lnw_p[:me] = ln_w
        lnb_p = np.zeros((e_pad, EEMB), dtype=np.float32)
        lnb_p[:me] = ln_b

    in_maps = []
    for cid in range(n_cores):
        sl = slice(cid * e_loc, (cid + 1) * e_loc)
        # edge e (local) = g*gch + j*128 + p  ->  rows[g, p, j]
        r = idx_p[0, sl].reshape(ng, kg, 128).transpose(0, 2, 1)
        ccol = idx_p[1, sl].reshape(ng, kg, 128).transpose(0, 2, 1)
        m = {
            "mx": mx,
            "mex": np.ascontiguousarray(mex_p[sl]),
            "rows": np.ascontiguousarray(r),
            "cols": np.ascontiguousarray(ccol),
            "w1": w1, "b1": b1, "w2": w2, "b2": b2, "w3": w3, "b3": b3,
        }
        if ln_affine:
            m["lnw"] = np.ascontiguousarray(lnw_p[sl])
            m["lnb"] = np.ascontiguousarray(lnb_p[sl])
        in_maps.append(m)
    return in_maps, ln_affine


_NC_CACHE = {}


def run_device(inputs, e_loc=40960, gch=2048, p2ch=1024, n_cores=NCORES,
               **run_kw):
    """Run the sharded kernel; returns the full [ME, 256] me_x_new plus the
    BassKernelResults (for profiling from test harnesses)."""
    in_maps, ln_affine = _prep_host(inputs, e_loc, gch, n_cores)
    mn = int(np.asarray(inputs["mx"]).shape[1])
    key = (e_loc, mn, gch, p2ch, ln_affine, n_cores)
    if key not in _NC_CACHE:
        _NC_CACHE[key] = build_nc(e_loc, mn, gch, p2ch, ln_affine, n_cores)
    nc = _NC_CACHE[key]
    res = run_bass_kernel_spmd(nc, in_maps, core_ids=list(range(n_cores)),
                               **run_kw)
    me = np.asarray(inputs["me_x"]).shape[1]
    full = np.concatenate([r["out"] for r in res.results], axis=0)[:me]
    return full, res


def kernel(**inputs):
    me_x = np.asarray(inputs["me_x"])
    full, _ = run_device(inputs)
    me_x_new = full[None].astype(np.float32)
    return (
        inputs["gx"],
        inputs["mx"],
        inputs["me_i"],
        me_x_new,
        inputs["g2me_i"],
        inputs["g2me_x"],
        inputs["m2ge_i"],
        inputs["m2ge_x"],
    )
